# revision 21
# baseline (speedup 1.0000x reference)
"""Trainium2 Bass kernel for DynamicFilterWithImageInput (v3: host filter
branch + column-split depthwise).

Model (per batch b):
  w   = softmax_over_C(BN2(mean_hw(relu(BN1(conv3x3(raw_img)))) @ w_filt.T + b_filt))
  out = depthwise_conv5x5(reflect_pad(x_feat), w.reshape(C,5,5))

The filter branch is tiny (~0.5 GFLOP for all 16 batches) and is computed
on the host in float64; only the 268 MB depthwise conv runs on device.

Per-core split of the 25 taps (i=row, j=col of the 5x5 kernel):
  - j in {0,4} (10 taps): full per-channel weights as DVE
    scalar_tensor_tensor MACs over an fp16 copy of x in [c,(y,x)] layout.
    STT hits the DVE 4x_2p perf mode (4 elem/cycle/lane) with all-fp16
    SBUF operands.
  - j in {1,2,3} (15 taps): channel-mean (wbar) via fp16 banded matmuls
    over the transposed [y,(x,c)] layout (3 band matrices per batch,
    vertical reflection folded in host-side), plus the per-channel
    residual dw = w - wbar:
      - 12 taps as fp8 DoubleRow diagonal matmuls on PE (2 taps/matmul),
      - 3 taps as DVE STT MACs.
  - PSUM evacuations ride the Scalar (ACT) engine, which also applies
    the 1/256 fp8 residual descale.

The PE issue rate is ~218 ns/matmul regardless of size (weight-load
bound), so the design minimizes matmul count: 12 resid + 6 rank matmuls
per 65k-output chunk vs 22 + 10 in the previous version.

Sharding: pure data-parallel over batch (16 batches -> 8 cores x 2).
Host adds the two partial outputs (produced in different layouts).
"""

import sys

sys.path.insert(0, "/opt/trn_rl_repo")

import numpy as np
import ml_dtypes

import concourse.bass as bass
import concourse.bacc as bacc
import concourse.mybir as mybir
import concourse.tile as tile
from concourse.bass_utils import run_bass_kernel_spmd

F8NP = ml_dtypes.float8_e4m3

F8 = mybir.dt.float8e4
F16 = mybir.dt.float16
F32 = mybir.dt.float32
ALU = mybir.AluOpType
DR = mybir.MatmulPerfMode.DoubleRow

EPS = 1e-5
B_PC = 2          # batches per core
C = 256           # channels
CG = C // 128     # channel groups of 128
K5 = 5
NSLAB = B_PC * CG
QR = 8            # output rows per residual quad
GR = 4            # rows per residual matmul group (psum-bank limit)
XG = 8            # x-cols per rank psum group
XPM = 4           # x-cols per rank matmul (N=512)
DW_SCALE = 256.0  # residual filter scale into fp8 range

JP = (1, 2, 3)    # kernel columns on the PE (banded + residual) path
JF = (0, 4)       # kernel columns fully on DVE
PE_NT = 12        # residual taps on PE (even; rest of the 15 go to DVE)

TAPS15 = [(i, j) for i in range(K5) for j in JP]
PE_TAPS = TAPS15[:PE_NT]
PE_PAIRS = [(PE_TAPS[2 * k], PE_TAPS[2 * k + 1]) for k in range(PE_NT // 2)]
DVE_TAPS = [(i, j) for j in JF for i in range(K5)] + TAPS15[PE_NT:]
NDVE = len(DVE_TAPS)
NJP = len(JP)

_PROG_CACHE = {}


def _as_strided(ap, dims, offset=None):
    n = ap.copy()
    v = n.ap
    v.clear()
    v.extend([list(d) for d in dims])
    if offset is not None:
        n.offset = offset
    return n


def _build_program(H, W):
    import os
    X16Q = os.environ.get("KV_X16Q", "scalar")
    ACC_TILE = bool(int(os.environ.get("KV_ACC_TILE", "0")))
    EVAC = os.environ.get("KV_EVAC", "act")
    ACC32 = bool(int(os.environ.get("KV_ACC32", "0")))
    DVE_DUMMY = bool(int(os.environ.get("KV_DVE_DUMMY", "0")))
    NTAP = int(os.environ.get("KV_NTAP", str(NDVE)))
    NO_TT = bool(int(os.environ.get("KV_NO_TT", "0")))
    TAPSEL = os.environ.get("KV_TAPSEL", "")
    SKIP_DVE = bool(int(os.environ.get("KV_SKIP_DVE", "0")))
    SKIP_RESID = bool(int(os.environ.get("KV_SKIP_RESID", "0")))
    SKIP_RANK = bool(int(os.environ.get("KV_SKIP_RANK", "0")))
    SKIP_WARM = bool(int(os.environ.get("KV_SKIP_WARM", "0")))
    Hp, Wp = H + 4, W + 4
    NQ = H // QR                  # 16 quads per slab
    NXG = W // XG                 # 16 rank groups per slab
    HH = H // 2                   # rows per half-slab
    HQ = NQ // 2                  # quads per half-slab
    HLEN = (HH + 6) * Wp          # padded rows needed per half (70*132)

    nc = bacc.Bacc("TRN2", target_bir_lowering=False, debug=False)

    x8_d = nc.dram_tensor("x8", [NSLAB, 128, Hp * Wp], F8, kind="ExternalInput").ap()
    x16_d = nc.dram_tensor("x16", [NSLAB, 128, Hp * Wp], F16, kind="ExternalInput").ap()
    xt_d = nc.dram_tensor("xt", [NSLAB, 128, Wp * 128], F16, kind="ExternalInput").ap()
    A_d = nc.dram_tensor("Ab", [B_PC, 128, NJP * 128], F16, kind="ExternalInput").ap()
    dts_d = nc.dram_tensor("dts", [NSLAB, 128, PE_NT * 128], F8, kind="ExternalInput").ap()
    wsc_d = nc.dram_tensor("wsc", [NSLAB, 128, NDVE], F32, kind="ExternalInput").ap()
    out_d = nc.dram_tensor("out", [B_PC, C, H, W], F16, kind="ExternalOutput").ap()
    y_d = nc.dram_tensor("yrk", [NSLAB, 128, W, 128], F16, kind="ExternalOutput").ap()
    eres_d = nc.dram_tensor("eres", [NSLAB, 128, H * W], F8, kind="ExternalOutput").ap()

    with tile.TileContext(nc) as tc:
        with (
            tc.tile_pool(name="consts", bufs=1) as consts,
            tc.tile_pool(name="x8p", bufs=2) as x8p,
            tc.tile_pool(name="x16p", bufs=2) as x16p,
            tc.tile_pool(name="xtp", bufs=int(os.environ.get("KV_XTBUF", "2"))) as xtp,
            tc.tile_pool(name="resp", bufs=2) as resp,
            tc.tile_pool(name="ot2p", bufs=3) as ot2p,
            tc.tile_pool(name="psA", bufs=2, space="PSUM") as psAp,
            tc.tile_pool(name="psB", bufs=2, space="PSUM") as psBp,
        ):
            # ---------- input loads (start streaming immediately) ----------
            x8s = [None] * NSLAB
            x16s = {}
            xts = [None] * NSLAB

            def load_x8(s):
                t = x8p.tile([128, Hp * Wp], F8, tag="x8")
                nc.scalar.dma_start(t[:], x8_d[s])
                x8s[s] = t

            def load_x16h(s, h):
                # 8 spare tail elems so shifted slice views stay in bounds
                t = x16p.tile([128, HLEN + 8], F16, tag="x16")
                o0 = 0 if h == 0 else (HH - 2) * Wp
                eng = nc.sync if X16Q == "sync" else nc.scalar
                eng.dma_start(t[:, 0:HLEN], x16_d[s][:, o0:o0 + HLEN])
                x16s[(s, h)] = t

            def load_xt(s):
                t = xtp.tile([128, Wp, 128], F16, tag="xt")
                nc.scalar.dma_start(
                    t[:], xt_d[s].rearrange("p (a b) -> p a b", a=Wp, b=128))
                xts[s] = t

            load_x8(0)
            load_x16h(0, 0)
            load_x16h(0, 1)
            load_xt(0)

            # ---------- constants ----------
            A_t = []
            for b in range(B_PC):
                At = consts.tile([128, NJP, 128], F16, tag=f"A{b}")
                nc.sync.dma_start(
                    At[:], A_d[b].rearrange("p (a b) -> p a b", a=NJP, b=128))
                A_t.append(At)
            dts_t = []
            for s in range(NSLAB):
                Dt = consts.tile([128, PE_NT, 128], F8, tag=f"D{s}")
                nc.sync.dma_start(
                    Dt[:], dts_d[s].rearrange("p (a b) -> p a b", a=PE_NT, b=128))
                dts_t.append(Dt)
            wsc_t = []
            for s in range(NSLAB):
                Wt = consts.tile([128, NDVE], F32, tag=f"W{s}")
                nc.sync.dma_start(Wt[:], wsc_d[s])
                wsc_t.append(Wt)

            # PE p-state warmup while the input DMAs stream (results unused)
            warm = consts.tile([128, 640], F16, tag="warm")
            nc.gpsimd.memset(warm[:], 0.0)
            for _ in range(0 if SKIP_WARM else 16):
                psw = psBp.tile([128, XG, 128], F32, tag="psB")
                nc.tensor.matmul(
                    psw[:, 0:XPM, :], warm[:, 0:128], warm[:, 128:640],
                    start=True, stop=True)

            # ---------- main loop ----------
            for s in range(NSLAB):
                b, cg = divmod(s, CG)
                x8 = x8s[s]
                for h in range(2):
                    res = resp.tile([128, HH * W], F16, tag="res")
                    res3 = res[:].rearrange("p (a b) -> p a b", a=HH, b=W)
                    eres = resp.tile([128, HH * W], F8, tag="eres", bufs=2)

                    # residual quads (PE fp8 DoubleRow) + ACT evac into eres
                    for q in range(0 if SKIP_RESID else HQ):
                        y0 = h * HH + q * QR
                        ps = psAp.tile([128, QR, W], F32, tag="psA")
                        for k, (ta, tb) in enumerate(PE_PAIRS):
                            ia, ja = ta
                            delta = (tb[0] - ta[0]) * Wp + (tb[1] - ta[1])
                            for g in range(QR // GR):
                                rhs = _as_strided(
                                    x8[:],
                                    [[Hp * Wp, 128], [delta, 2], [Wp, GR], [1, W]],
                                    (y0 + ia) * Wp + ja + g * GR * Wp,
                                )
                                nc.tensor.matmul(
                                    ps[:, g * GR:(g + 1) * GR, :],
                                    dts_t[s][:, 2 * k:2 * k + 2, :], rhs,
                                    start=(k == 0), stop=(k == len(PE_PAIRS) - 1),
                                    perf_mode=DR,
                                )
                        nc.scalar.copy(
                            eres[:, q * QR * W:(q + 1) * QR * W].rearrange(
                                "p (a b) -> p a b", a=QR, b=W),
                            ps[:])

                    if h == 1:
                        if s + 1 < NSLAB:
                            load_x8(s + 1)

                    # rank groups (PE fp16 banded) + ACT evac
                    for xg in (() if SKIP_RANK else range(h * HQ, h * HQ + HQ)):
                        x0 = xg * XG
                        ps2 = psBp.tile([128, XG, 128], F32, tag="psB")
                        ot2 = ot2p.tile([128, XG, 128], F16, tag="ot2")
                        for jidx, j in enumerate(JP):
                            for xm in range(XG // XPM):
                                rhs = _as_strided(
                                    xts[s][:],
                                    [[Wp * 128, 128], [128, XPM], [1, 128]],
                                    (x0 + xm * XPM + j) * 128,
                                )
                                nc.tensor.matmul(
                                    ps2[:, xm * XPM:(xm + 1) * XPM, :],
                                    A_t[b][:, jidx, :], rhs,
                                    start=(jidx == 0), stop=(jidx == NJP - 1),
                                )
                        nc.scalar.copy(ot2[:], ps2[:])
                        nc.sync.dma_start(y_d[s][:, x0:x0 + XG, :], ot2[:])

                    # DVE taps: first tap overwrites res (DMA-gated chain head),
                    # rest accumulate in place; PE part merged by one TT add.
                    x16h = x16s[(s, h)]
                    for t, (i, j) in list(enumerate(DVE_TAPS))[:NTAP]:
                        base = (i + (2 if h == 1 else 0)) * Wp + j
                        in0 = x16h[:, base:base + HH * Wp].rearrange(
                            "p (a b) -> p a b", a=HH, b=Wp)[:, :, 0:W]
                        if t == 0:
                            nc.vector.tensor_scalar(
                                res3, in0, wsc_t[s][:, t:t + 1], None, ALU.mult)
                        else:
                            nc.vector.scalar_tensor_tensor(
                                res3, in0, wsc_t[s][:, t:t + 1], res3,
                                ALU.mult, ALU.add)
                    if not SKIP_RESID:
                        nc.sync.dma_start(
                            eres_d[s][:, h * HH * W:(h + 1) * HH * W], eres[:])
                    nc.sync.dma_start(
                        out_d[b, cg * 128:(cg + 1) * 128,
                              h * HH:(h + 1) * HH, :], res3)
                    # prefetch next users of this x16 slot
                    if s + 1 < NSLAB:
                        load_x16h(s + 1, h)
                # prefetch next slab's xt after its 16 rank groups consumed it
                if s + 1 < NSLAB:
                    load_xt(s + 1)

    nc.compile()
    return nc


def get_program(H, W):
    key = (H, W)
    if key not in _PROG_CACHE:
        _PROG_CACHE[key] = _build_program(H, W)
    return _PROG_CACHE[key]


def _reflect_idx(r, n):
    if r < 0:
        return -r
    if r >= n:
        return 2 * n - 2 - r
    return r


def host_filter_branch(raw_img, w_conv1, b_conv1, g1, beta1, m1, v1,
                       w_filt, b_filt, g2, beta2, m2, v2):
    """Exact (float64) replica of the reference filter branch -> w (B,C,25)."""
    B = raw_img.shape[0]
    H, W = raw_img.shape[2], raw_img.shape[3]
    rawpad = np.pad(np.asarray(raw_img, np.float64),
                    ((0, 0), (0, 0), (1, 1), (1, 1)))
    w1 = np.asarray(w_conv1, np.float64)
    y = np.zeros((B, 64, H, W), np.float64)
    for i in range(3):
        for j in range(3):
            y += np.einsum("oc,bchw->bohw", w1[:, :, i, j],
                           rawpad[:, :, i:i + H, j:j + W])
    y = y + np.asarray(b_conv1, np.float64)[None, :, None, None]
    a1 = np.asarray(g1, np.float64) / np.sqrt(np.asarray(v1, np.float64) + EPS)
    y = (y - np.asarray(m1, np.float64)[None, :, None, None]) * a1[None, :, None, None] \
        + np.asarray(beta1, np.float64)[None, :, None, None]
    y = np.maximum(y, 0.0)
    img_feat = y.mean(axis=(2, 3))                      # (B, 64)

    df = img_feat @ np.asarray(w_filt, np.float64).T + np.asarray(b_filt, np.float64)
    a2 = np.asarray(g2, np.float64) / np.sqrt(np.asarray(v2, np.float64) + EPS)
    df = (df - np.asarray(m2, np.float64)) * a2 + np.asarray(beta2, np.float64)
    df = df.reshape(B, C, K5 * K5)
    df = df - df.max(axis=1, keepdims=True)
    e = np.exp(df)
    return e / e.sum(axis=1, keepdims=True)             # (B, C, 25)


def host_prep(x_feat, raw_img, w_conv1, b_conv1, g1, beta1, m1, v1,
              w_filt, b_filt, g2, beta2, m2, v2):
    B, Cc, H, W = x_feat.shape
    assert Cc == C
    n_cores = B // B_PC
    Hp, Wp = H + 4, W + 4

    w = host_filter_branch(raw_img, w_conv1, b_conv1, g1, beta1, m1, v1,
                           w_filt, b_filt, g2, beta2, m2, v2)
    wk = w.reshape(B, C, K5, K5)
    wbar = wk.mean(axis=1)                              # (B, 5, 5)
    dw = wk - wbar[:, None]                             # (B, C, 5, 5)

    # band matrices for the JP columns (vertical reflection folded in)
    Ah = np.zeros((B, 128, NJP, 128), np.float64)
    for jidx, j in enumerate(JP):
        for i in range(K5):
            for yout in range(H):
                yin = _reflect_idx(yout + i - 2, H)
                Ah[:, yin, jidx, yout] += wbar[:, i, j]
    A16 = Ah.astype(np.float16).reshape(B, 128, NJP * 128)

    # fp8 DoubleRow diagonal pair tiles for the PE residual taps
    dq = (dw * DW_SCALE).astype(np.float16).astype(F8NP)
    dqr = dq.reshape(B, CG, 128, K5, K5)
    D = np.zeros((B, CG, 128, PE_NT, 128), F8NP)
    cidx = np.arange(128)
    for k, (ta, tb) in enumerate(PE_PAIRS):
        D[:, :, cidx, 2 * k, cidx] = dqr[:, :, cidx, ta[0], ta[1]]
        D[:, :, cidx, 2 * k + 1, cidx] = dqr[:, :, cidx, tb[0], tb[1]]

    # DVE per-channel scalars: 10 full-weight taps + leftover residual taps
    wkr = wk.reshape(B, CG, 128, K5, K5)
    dwr = dw.reshape(B, CG, 128, K5, K5)
    wscv = np.zeros((B, CG, 128, NDVE), np.float32)
    for t, (i, j) in enumerate(DVE_TAPS):
        src = wkr if j in JF else dwr
        wscv[:, :, :, t] = src[:, :, :, i, j]

    # x streams
    xpad16 = np.pad(x_feat, ((0, 0), (0, 0), (2, 2), (2, 2)),
                    mode="reflect").astype(np.float16)
    x16 = xpad16.reshape(B, CG, 128, Hp * Wp)
    x8 = xpad16.astype(F8NP).reshape(B, CG, 128, Hp * Wp)

    # transposed (cols-only padded) layout per slab: [y, (x, c)]
    xpc = np.pad(x_feat, ((0, 0), (0, 0), (0, 0), (2, 2)),
                 mode="reflect").astype(np.float16)          # (B, C, H, Wp)
    xt = np.ascontiguousarray(
        xpc.reshape(B, CG, 128, H, Wp).transpose(0, 1, 3, 4, 2)
    ).reshape(B, CG, 128, Wp * 128)                          # [b, cg, y, (x c)]

    in_maps = []
    for core in range(n_cores):
        bs = core * B_PC
        in_maps.append({
            "x8": np.ascontiguousarray(x8[bs:bs + B_PC]).reshape(
                NSLAB, 128, Hp * Wp),
            "x16": np.ascontiguousarray(x16[bs:bs + B_PC]).reshape(
                NSLAB, 128, Hp * Wp),
            "xt": np.ascontiguousarray(xt[bs:bs + B_PC]).reshape(
                NSLAB, 128, Wp * 128),
            "Ab": np.ascontiguousarray(A16[bs:bs + B_PC]),
            "dts": np.ascontiguousarray(D[bs:bs + B_PC]).reshape(
                NSLAB, 128, PE_NT * 128),
            "wsc": np.ascontiguousarray(wscv[bs:bs + B_PC]).reshape(
                NSLAB, 128, NDVE),
        })
    return in_maps


def run(inputs, trace=False, **_ignored):
    x_feat = inputs["x_feat"]
    B, _, H, W = x_feat.shape
    nc = get_program(H, W)
    in_maps = host_prep(**inputs)
    n_cores = len(in_maps)
    res = run_bass_kernel_spmd(nc, in_maps, list(range(n_cores)), trace=trace)
    outs = []
    for r in res.results:
        resid = r["out"].astype(np.float32)            # (B_PC, C, H, W)
        er = r["eres"].astype(np.float32) * (1.0 / DW_SCALE)
        resid = resid + er.reshape(B_PC, C, H, W)
        yr = r["yrk"].astype(np.float32)               # (NSLAB, 128, W, 128)
        rank = yr.reshape(B_PC, CG, 128, W, 128).transpose(0, 1, 4, 2, 3)
        rank = rank.reshape(B_PC, C, H, W)
        outs.append(resid + rank)
    out = np.concatenate(outs, axis=0)
    return out, res


def kernel(**inputs) -> np.ndarray:
    out, _ = run(inputs, trace=False)
    return out


# revision 25
# speedup vs baseline: 1.8998x; 1.8998x over previous
"""Trainium2 Bass kernel for DynamicFilterWithImageInput (v4: host filter
branch + wbar banded rank + split residual, host-merged outputs).

Model (per batch b):
  w   = softmax_over_C(BN2(mean_hw(relu(BN1(conv3x3(raw_img)))) @ w_filt.T + b_filt))
  out = depthwise_conv5x5(reflect_pad(x_feat), w.reshape(C,5,5))

The filter branch is tiny (~0.5 GFLOP for all 16 batches) and is computed
on the host in float64; only the 268 MB depthwise conv runs on device.

Decomposition: w[c,t] = wbar[t] + dw[c,t], wbar = channel mean.
  - wbar part (all 25 taps): fp16 banded matmuls over the transposed
    [y,(x,c)] layout; 5 band matrices per batch (vertical reflection
    folded in host-side), 5 matmuls per 512-column chunk -> "yrk" output.
  - dw residual, 17 taps: fp8 DoubleRow diagonal matmuls on PE
    (2 taps/matmul, 9 slots), ACT evacuates PSUM (x 1/256) -> "eres".
  - dw residual, 8 taps: custom DVE PAIR_MAC ops over the fp8 x in
    [c,(y,x)] layout (2 taps/pass) -> "dres".
  The three partial outputs are summed on the host (different layouts).
  Cross-engine merges on device are avoided deliberately: DVE
  instructions consuming ACT-written tiles deadlock this hardware.

PE issue rate is ~218 ns/matmul regardless of size (weight-load bound),
so matmul count is the budget: 18 resid + 10 rank per 65k-output chunk.

Sharding: pure data-parallel over batch (16 batches -> 8 cores x 2).
"""

import sys

sys.path.insert(0, "/opt/trn_rl_repo")

import numpy as np
import ml_dtypes

import concourse.bass as bass
import concourse.bacc as bacc
import concourse.mybir as mybir
import concourse.tile as tile
from concourse.bass_utils import run_bass_kernel_spmd
import concourse.dve_ops as _dve_ops


def _get_pair_mac():
    """Fused custom DVE op: out = in0*s0 + in1*s1 (two conv taps per pass)."""
    if hasattr(_dve_ops, "PAIR_MAC_ANT"):
        return _dve_ops.PAIR_MAC_ANT
    from concourse.dve_spec import Spec, Src0, Src1, C0, C1
    op = _dve_ops.DveOp(
        "PAIR_MAC_ANT",
        Spec(
            body=Src0 * C0 + Src1 * C1,
            reference=lambda in0, in1, s0, s1, imm2: (
                in0.astype(np.float32) * s0 + in1.astype(np.float32) * s1
            ).astype(np.float32),
        ),
        subdim=False,
        uops_sha={"v3": "f2ac165a27dbafb3", "v4": "49eb47656a95aba3"},
    )
    _dve_ops.OPS.append(op)
    _dve_ops.CUSTOM_DVE_SPECS[op.name] = op.spec
    _dve_ops._SUB_OPCODE_FOR_NAME[op.name] = (
        _dve_ops._CUSTOM_DVE_ROW_BASE + len(_dve_ops.OPS) - 1
    )
    assert max(_dve_ops._SUB_OPCODE_FOR_NAME.values()) < 0x20
    _dve_ops.PAIR_MAC_ANT = op
    return op


PAIR_MAC_ANT = _get_pair_mac()

F8NP = ml_dtypes.float8_e4m3

F8 = mybir.dt.float8e4
F16 = mybir.dt.float16
F32 = mybir.dt.float32
ALU = mybir.AluOpType
DR = mybir.MatmulPerfMode.DoubleRow

EPS = 1e-5
B_PC = 2          # batches per core
C = 256           # channels
CG = C // 128     # channel groups of 128
K5 = 5
NSLAB = B_PC * CG
QR = 8            # output rows per residual quad
GR = 4            # rows per residual matmul group (psum-bank limit)
XG = 8            # x-cols per rank psum group
XPM = 4           # x-cols per rank matmul (N=512)
DW_SCALE = 256.0  # residual filter scale into fp8 range

# residual tap split: first NDVE row-major taps on DVE customs, rest on PE
ALL_TAPS = [(i, j) for i in range(K5) for j in range(K5)]
NDVE = 4
DVE_TAPS = ALL_TAPS[:NDVE]
DVE_PAIRS = [(DVE_TAPS[2 * k], DVE_TAPS[2 * k + 1]) for k in range(NDVE // 2)]
PE_TAPS = ALL_TAPS[NDVE:]
PE_PAIRS = [(PE_TAPS[2 * k], PE_TAPS[2 * k + 1])
            for k in range(len(PE_TAPS) // 2)]
if len(PE_TAPS) % 2:
    PE_PAIRS.append((PE_TAPS[-1], None))
NSLOT = len(PE_PAIRS)

_PROG_CACHE = {}


def _as_strided(ap, dims, offset=None):
    n = ap.copy()
    v = n.ap
    v.clear()
    v.extend([list(d) for d in dims])
    if offset is not None:
        n.offset = offset
    return n


def _build_program(H, W):
    Hp, Wp = H + 4, W + 4
    NQ = H // QR                  # 16 quads per slab
    HH = H // 2                   # rows per half-slab
    HQ = NQ // 2                  # quads per half-slab
    XFREE = Hp * Wp + 8           # x8 tile pitch (spare tail for flat runs)

    nc = bacc.Bacc("TRN2", target_bir_lowering=False, debug=False)

    x8_d = nc.dram_tensor("x8", [NSLAB, 128, Hp * Wp], F8, kind="ExternalInput").ap()
    xt_d = nc.dram_tensor("xt", [NSLAB, 128, Wp * 128], F16, kind="ExternalInput").ap()
    A_d = nc.dram_tensor("Ab", [B_PC, 128, K5 * 128], F16, kind="ExternalInput").ap()
    dts_d = nc.dram_tensor("dts", [NSLAB, 128, NSLOT * 2 * 128], F8,
                           kind="ExternalInput").ap()
    wsc_d = nc.dram_tensor("wsc", [NSLAB, 128, NDVE], F32, kind="ExternalInput").ap()
    y_d = nc.dram_tensor("yrk", [NSLAB, 128, W, 128], F16, kind="ExternalOutput").ap()
    eres_d = nc.dram_tensor("eres", [NSLAB, 128, H * W], F16,
                            kind="ExternalOutput").ap()
    dres_d = nc.dram_tensor("dres", [NSLAB, 128, H * W], F16,
                            kind="ExternalOutput").ap()

    with tile.TileContext(nc) as tc:
        with (
            tc.tile_pool(name="consts", bufs=1) as consts,
            tc.tile_pool(name="x8p", bufs=2) as x8p,
            tc.tile_pool(name="xtp", bufs=2) as xtp,
            tc.tile_pool(name="qsp", bufs=2) as qsp,
            tc.tile_pool(name="eresp", bufs=2) as eresp,
            tc.tile_pool(name="ot2p", bufs=3) as ot2p,
            tc.tile_pool(name="psA", bufs=2, space="PSUM") as psAp,
            tc.tile_pool(name="psB", bufs=2, space="PSUM") as psBp,
        ):
            # ---------- input loads (start streaming immediately) ----------
            x8s = [None] * NSLAB
            xts = [None] * NSLAB

            def load_x8(s):
                t = x8p.tile([128, XFREE], F8, tag="x8")
                nc.vector.memset(t[:, Hp * Wp:], 0.0)
                nc.scalar.dma_start(t[:, 0:Hp * Wp], x8_d[s])
                x8s[s] = t

            def load_xt(s):
                t = xtp.tile([128, Wp, 128], F16, tag="xt")
                nc.scalar.dma_start(
                    t[:], xt_d[s].rearrange("p (a b) -> p a b", a=Wp, b=128))
                xts[s] = t

            load_x8(0)
            load_xt(0)
            load_x8(1)

            # ---------- constants ----------
            A_t = []
            for b in range(B_PC):
                At = consts.tile([128, K5, 128], F16, tag=f"A{b}")
                nc.sync.dma_start(
                    At[:], A_d[b].rearrange("p (a b) -> p a b", a=K5, b=128))
                A_t.append(At)
            dts_t = []
            for s in range(NSLAB):
                Dt = consts.tile([128, NSLOT * 2, 128], F8, tag=f"D{s}")
                nc.sync.dma_start(
                    Dt[:], dts_d[s].rearrange("p (a b) -> p a b",
                                              a=NSLOT * 2, b=128))
                dts_t.append(Dt)
            wsc_t = []
            for s in range(NSLAB):
                Wt = consts.tile([128, NDVE], F32, tag=f"W{s}")
                nc.sync.dma_start(Wt[:], wsc_d[s])
                wsc_t.append(Wt)

            # PE p-state warmup while the input DMAs stream (results unused)
            warm = consts.tile([128, 640], F16, tag="warm")
            nc.gpsimd.memset(warm[:], 0.0)
            for _ in range(16):
                psw = psBp.tile([128, XG, 128], F32, tag="psB")
                nc.tensor.matmul(
                    psw[:, 0:XPM, :], warm[:, 0:128], warm[:, 128:640],
                    start=True, stop=True)

            # ---------- main loop ----------
            for s in range(NSLAB):
                b, cg = divmod(s, CG)
                x8 = x8s[s]
                for h in range(2):
                    eres = eresp.tile([128, HH * W], F16, tag="eres")

                    # residual quads (PE fp8 DoubleRow) + ACT evac to eres
                    for q in range(HQ):
                        y0 = h * HH + q * QR
                        ps = psAp.tile([128, QR, W], F32, tag="psA")
                        for k, (ta, tb) in enumerate(PE_PAIRS):
                            ia, ja = ta
                            tb_ = tb if tb is not None else ta
                            delta = max((tb_[0] - ta[0]) * Wp + (tb_[1] - ta[1]), 1)
                            for g in range(QR // GR):
                                rhs = _as_strided(
                                    x8[:],
                                    [[XFREE, 128], [delta, 2], [Wp, GR], [1, W]],
                                    (y0 + ia) * Wp + ja + g * GR * Wp,
                                )
                                nc.tensor.matmul(
                                    ps[:, g * GR:(g + 1) * GR, :],
                                    dts_t[s][:, 2 * k:2 * k + 2, :], rhs,
                                    start=(k == 0), stop=(k == NSLOT - 1),
                                    perf_mode=DR,
                                )
                        nc.scalar.mul(
                            eres[:, q * QR * W:(q + 1) * QR * W].rearrange(
                                "p (a b) -> p a b", a=QR, b=W),
                            ps[:], 1.0 / DW_SCALE)
                    nc.sync.dma_start(
                        eres_d[s][:, h * HH * W:(h + 1) * HH * W], eres[:])

                    if h == 1 and s + 2 < NSLAB:
                        load_x8(s + 2)

                    # rank groups (PE fp16 banded, all 5 columns) + ACT evac
                    for xg in range(h * HQ, h * HQ + HQ):
                        x0 = xg * XG
                        ps2 = psBp.tile([128, XG, 128], F32, tag="psB")
                        ot2 = ot2p.tile([128, XG, 128], F16, tag="ot2")
                        for j in range(K5):
                            for xm in range(XG // XPM):
                                rhs = _as_strided(
                                    xts[s][:],
                                    [[Wp * 128, 128], [128, XPM], [1, 128]],
                                    (x0 + xm * XPM + j) * 128,
                                )
                                nc.tensor.matmul(
                                    ps2[:, xm * XPM:(xm + 1) * XPM, :],
                                    A_t[b][:, j, :], rhs,
                                    start=(j == 0), stop=(j == K5 - 1),
                                )
                        nc.scalar.copy(ot2[:], ps2[:])
                        nc.sync.dma_start(y_d[s][:, x0:x0 + XG, :], ot2[:])

                    # DVE residual taps: custom PAIR_MAC over x8 half-runs
                    def run_ap(i, j):
                        off = (h * HH + i) * Wp + j
                        return x8[:, off:off + HH * Wp]

                    qs0 = qsp.tile([128, HH * Wp], F16, tag="qs0")
                    for k, (ta, tb) in enumerate(DVE_PAIRS):
                        tgt = qs0 if k == 0 else qsp.tile(
                            [128, HH * Wp], F16, tag="qtmp", bufs=1)
                        nc.vector._custom_dve(
                            PAIR_MAC_ANT, out=tgt[:],
                            in0=run_ap(*ta), in1=run_ap(*tb),
                            s0=wsc_t[s][:, 2 * k:2 * k + 1],
                            s1=wsc_t[s][:, 2 * k + 1:2 * k + 2])
                        if k > 0:
                            nc.vector.tensor_tensor(
                                qs0[:], qs0[:], tgt[:], ALU.add)
                    nc.sync.dma_start(
                        dres_d[s][:, h * HH * W:(h + 1) * HH * W].rearrange(
                            "p (a b) -> p a b", a=HH, b=W),
                        qs0[:].rearrange(
                            "p (a b) -> p a b", a=HH, b=Wp)[:, :, 0:W])
                # prefetch next slab's xt after its rank groups consumed it
                if s + 1 < NSLAB:
                    load_xt(s + 1)

    nc.compile()
    return nc


def get_program(H, W):
    key = (H, W)
    if key not in _PROG_CACHE:
        _PROG_CACHE[key] = _build_program(H, W)
    return _PROG_CACHE[key]


def _reflect_idx(r, n):
    if r < 0:
        return -r
    if r >= n:
        return 2 * n - 2 - r
    return r


def host_filter_branch(raw_img, w_conv1, b_conv1, g1, beta1, m1, v1,
                       w_filt, b_filt, g2, beta2, m2, v2):
    """Exact (float64) replica of the reference filter branch -> w (B,C,25)."""
    B = raw_img.shape[0]
    H, W = raw_img.shape[2], raw_img.shape[3]
    rawpad = np.pad(np.asarray(raw_img, np.float64),
                    ((0, 0), (0, 0), (1, 1), (1, 1)))
    w1 = np.asarray(w_conv1, np.float64)
    y = np.zeros((B, 64, H, W), np.float64)
    for i in range(3):
        for j in range(3):
            y += np.einsum("oc,bchw->bohw", w1[:, :, i, j],
                           rawpad[:, :, i:i + H, j:j + W])
    y = y + np.asarray(b_conv1, np.float64)[None, :, None, None]
    a1 = np.asarray(g1, np.float64) / np.sqrt(np.asarray(v1, np.float64) + EPS)
    y = (y - np.asarray(m1, np.float64)[None, :, None, None]) * a1[None, :, None, None] \
        + np.asarray(beta1, np.float64)[None, :, None, None]
    y = np.maximum(y, 0.0)
    img_feat = y.mean(axis=(2, 3))                      # (B, 64)

    df = img_feat @ np.asarray(w_filt, np.float64).T + np.asarray(b_filt, np.float64)
    a2 = np.asarray(g2, np.float64) / np.sqrt(np.asarray(v2, np.float64) + EPS)
    df = (df - np.asarray(m2, np.float64)) * a2 + np.asarray(beta2, np.float64)
    df = df.reshape(B, C, K5 * K5)
    df = df - df.max(axis=1, keepdims=True)
    e = np.exp(df)
    return e / e.sum(axis=1, keepdims=True)             # (B, C, 25)


def host_prep(x_feat, raw_img, w_conv1, b_conv1, g1, beta1, m1, v1,
              w_filt, b_filt, g2, beta2, m2, v2):
    B, Cc, H, W = x_feat.shape
    assert Cc == C
    n_cores = B // B_PC
    Hp, Wp = H + 4, W + 4

    w = host_filter_branch(raw_img, w_conv1, b_conv1, g1, beta1, m1, v1,
                           w_filt, b_filt, g2, beta2, m2, v2)
    wk = w.reshape(B, C, K5, K5)
    wbar = wk.mean(axis=1)                              # (B, 5, 5)
    dw = wk - wbar[:, None]                             # (B, C, 5, 5)

    # band matrices for all 5 columns (vertical reflection folded in)
    Ah = np.zeros((B, 128, K5, 128), np.float64)
    for j in range(K5):
        for i in range(K5):
            for yout in range(H):
                yin = _reflect_idx(yout + i - 2, H)
                Ah[:, yin, j, yout] += wbar[:, i, j]
    A16 = Ah.astype(np.float16).reshape(B, 128, K5 * 128)

    # fp8 DoubleRow diagonal pair tiles for the PE residual taps
    dq = (dw * DW_SCALE).astype(np.float16).astype(F8NP)
    dqr = dq.reshape(B, CG, 128, K5, K5)
    D = np.zeros((B, CG, 128, NSLOT * 2, 128), F8NP)
    cidx = np.arange(128)
    for k, (ta, tb) in enumerate(PE_PAIRS):
        D[:, :, cidx, 2 * k, cidx] = dqr[:, :, cidx, ta[0], ta[1]]
        if tb is not None:
            D[:, :, cidx, 2 * k + 1, cidx] = dqr[:, :, cidx, tb[0], tb[1]]

    # DVE per-channel residual scalars (true units; x8 is true-unit fp8)
    dwr = dw.reshape(B, CG, 128, K5, K5)
    wscv = np.zeros((B, CG, 128, NDVE), np.float32)
    for t, (i, j) in enumerate(DVE_TAPS):
        wscv[:, :, :, t] = dwr[:, :, :, i, j]

    # x streams
    xpad16 = np.pad(x_feat, ((0, 0), (0, 0), (2, 2), (2, 2)),
                    mode="reflect").astype(np.float16)
    x8 = xpad16.astype(F8NP).reshape(B, CG, 128, Hp * Wp)

    # transposed (cols-only padded) layout per slab: [y, (x, c)]
    xpc = np.pad(x_feat, ((0, 0), (0, 0), (0, 0), (2, 2)),
                 mode="reflect").astype(np.float16)          # (B, C, H, Wp)
    xt = np.ascontiguousarray(
        xpc.reshape(B, CG, 128, H, Wp).transpose(0, 1, 3, 4, 2)
    ).reshape(B, CG, 128, Wp * 128)                          # [b, cg, y, (x c)]

    in_maps = []
    for core in range(n_cores):
        bs = core * B_PC
        in_maps.append({
            "x8": np.ascontiguousarray(x8[bs:bs + B_PC]).reshape(
                NSLAB, 128, Hp * Wp),
            "xt": np.ascontiguousarray(xt[bs:bs + B_PC]).reshape(
                NSLAB, 128, Wp * 128),
            "Ab": np.ascontiguousarray(A16[bs:bs + B_PC]),
            "dts": np.ascontiguousarray(D[bs:bs + B_PC]).reshape(
                NSLAB, 128, NSLOT * 2 * 128),
            "wsc": np.ascontiguousarray(wscv[bs:bs + B_PC]).reshape(
                NSLAB, 128, NDVE),
        })
    return in_maps


def run(inputs, trace=False, **_ignored):
    x_feat = inputs["x_feat"]
    B, _, H, W = x_feat.shape
    nc = get_program(H, W)
    in_maps = host_prep(**inputs)
    n_cores = len(in_maps)
    res = run_bass_kernel_spmd(nc, in_maps, list(range(n_cores)), trace=trace)
    outs = []
    for r in res.results:
        resid = (r["eres"].astype(np.float32) + r["dres"].astype(np.float32))
        resid = resid.reshape(B_PC, C, H, W)
        yr = r["yrk"].astype(np.float32)               # (NSLAB, 128, W, 128)
        rank = yr.reshape(B_PC, CG, 128, W, 128).transpose(0, 1, 4, 2, 3)
        rank = rank.reshape(B_PC, C, H, W)
        outs.append(resid + rank)
    out = np.concatenate(outs, axis=0)
    return out, res


def kernel(**inputs) -> np.ndarray:
    out, _ = run(inputs, trace=False)
    return out
